# revision 22
# baseline (speedup 1.0000x reference)
"""Trainium2 Bass kernel for nn_Att_H_47571057771075.

Data-parallel over batch: 64 batches -> 8 cores x 8 batches; on each core the
8 batches are processed as 4 pairs so every elementwise/matmul instruction
covers two batches (amortizes per-instruction fixed costs).

Performance design:
- float32r end-to-end for every matmul operand (1 cycle/row on the PE for
  moving dim >= 256, vs 4 for plain fp32).
- One ACT table set for the whole kernel (exp_and_others): sigmoid/silu come
  from tanh, softmax from exp; only the one-time numNeighbors sqrt uses
  another set.
- Q and K share one PE pass (stationary [Wq^T|Wk^T]); scores are computed
  transposed so exp runs straight off PSUM and A@V needs no transpose of A.
- sigmoid(x) = 0.5(1+tanh(x/2)): the +1 rank-1 terms of Q,K fold into an
  extra all-ones column of the scores matmul (per-key bias into exp) and are
  dropped for the per-query terms (softmax-invariant). Constant 0.5/0.25
  factors fold into host-side weights / the exp scale.
- Softmax denominator Z comes free from an all-ones stationary column in the
  A@V matmul; normalization is deferred through the linear output projection.
- Ones-columns live in persistent double-buffered tiles, written once.
"""
import os
import sys

for p in ("/opt/trn_rl_repo",):
    if p not in sys.path:
        sys.path.insert(0, p)

import numpy as np

import concourse.bacc as bacc
import concourse.tile as tile
from concourse import mybir
from concourse.masks import make_identity
from concourse.bass_utils import run_bass_kernel_spmd

F32 = mybir.dt.float32
F32R = mybir.dt.float32r
AF = mybir.ActivationFunctionType
ALU = mybir.AluOpType

B, D, NA, R, H = 64, 2048, 256, 64, 64
NCORES = 8
BL = B // NCORES          # batches per core
NP = BL // 2              # batch pairs per core
KC = D // 128             # k-chunks for layer 1 (16)

_cache = {}


def build_bass():
    nc = bacc.Bacc("TRN2", target_bir_lowering=False, debug=False)

    x_d = nc.dram_tensor("x", [NP, 128, KC, 2, NA], F32R, kind="ExternalInput")
    l_d = nc.dram_tensor("l0", [BL, NA], F32, kind="ExternalInput")
    wqk_d = nc.dram_tensor("wqk", [D, 128], F32R, kind="ExternalInput")
    wv_d = nc.dram_tensor("wv", [D, R], F32R, kind="ExternalInput")
    wqk1_d = nc.dram_tensor("wqk1", [R, 128], F32R, kind="ExternalInput")
    wv1_d = nc.dram_tensor("wv1", [R, R], F32R, kind="ExternalInput")
    wqk5_d = nc.dram_tensor("wqk5", [R, 128], F32R, kind="ExternalInput")
    wv5_d = nc.dram_tensor("wv5", [R, R], F32R, kind="ExternalInput")
    ao_d = nc.dram_tensor("aoT", [R, R], F32R, kind="ExternalInput")
    ao1_d = nc.dram_tensor("ao1T", [R, R], F32R, kind="ExternalInput")
    ao5_d = nc.dram_tensor("ao5T", [R, H], F32R, kind="ExternalInput")
    g_d = nc.dram_tensor("gmat", [H, 5], F32R, kind="ExternalInput")
    out_d = nc.dram_tensor("out", [1, BL], F32, kind="ExternalOutput")

    with tile.TileContext(nc) as tc:
        with (
            tc.tile_pool(name="singles", bufs=1) as singles,
            tc.tile_pool(name="xpool", bufs=2) as xpool,
            tc.tile_pool(name="work", bufs=2) as work,
            tc.tile_pool(name="small", bufs=2) as small,
            tc.tile_pool(name="pa", bufs=2, space="PSUM") as pa,
            tc.tile_pool(name="pb", bufs=2, space="PSUM") as pb,
            tc.tile_pool(name="pf", bufs=2, space="PSUM") as pfp,
        ):
            # ---- per-batch 0.25/sqrt(numN) first (one sqrt-table load) ----
            l_sb = small.tile([BL, NA], F32, tag="lsb")
            nc.sync.dma_start(out=l_sb, in_=l_d[:])
            ind = small.tile([BL, NA], F32, tag="ind")
            nc.vector.tensor_single_scalar(ind, l_sb, 1.0, ALU.is_ge)
            s8 = small.tile([BL, 1], F32, tag="s8")
            nc.vector.reduce_sum(s8, ind, axis=mybir.AxisListType.X)
            c16 = small.tile([BL, 1], F32, tag="c16")
            nc.vector.memset(c16, 16.0)
            nc.scalar.activation(s8, s8, AF.Sqrt, bias=c16, scale=16.0)
            inv8 = small.tile([BL, 1], F32, tag="inv8")
            nc.vector.reciprocal(inv8, s8)

            ident = singles.tile([128, 128], F32)
            make_identity(nc, ident[:])
            ones_sb = singles.tile([1, 128], F32)
            nc.vector.memset(ones_sb, 1.0)
            ones_f32 = singles.tile([128, R], F32)
            nc.vector.memset(ones_f32, 1.0)
            ones_tall = singles.tile([128, R], F32R)
            nc.scalar.activation(ones_tall, ones_f32, AF.Copy)
            onesq = singles.tile([R, 2, 2], F32)
            nc.vector.memset(onesq, 1.0)
            ones4 = singles.tile([128, 2, 2, 1], F32)
            nc.vector.memset(ones4, 1.0)
            ones5 = singles.tile([5, 1], F32)
            nc.vector.memset(ones5, 1.0)
            res_sb = singles.tile([1, BL], F32)

            pt = pb.tile([1, BL], F32, tag="pbx")
            nc.tensor.transpose(pt, inv8, ident[0:BL, 0:BL])
            invrow = small.tile([1, BL], F32, tag="invrow")
            nc.vector.tensor_copy(invrow, pt)
            pib = pb.tile([128, BL], F32, tag="pbx")
            nc.tensor.matmul(pib, ones_sb, invrow)
            invb4 = singles.tile([128, BL], F32)
            nc.vector.tensor_copy(invb4, pib)

            # ---- weights ----
            wqk_sb = singles.tile([128, KC, 128], F32R)
            nc.sync.dma_start(out=wqk_sb, in_=wqk_d[:].rearrange("(c p) m -> p c m", p=128))
            wv_sb = singles.tile([128, KC, R], F32R)
            nc.sync.dma_start(out=wv_sb, in_=wv_d[:].rearrange("(c p) m -> p c m", p=128))
            wqk1_sb = singles.tile([R, 128], F32R)
            nc.sync.dma_start(out=wqk1_sb, in_=wqk1_d[:])
            wv1_sb = singles.tile([R, R], F32R)
            nc.sync.dma_start(out=wv1_sb, in_=wv1_d[:])
            wqk5_sb = singles.tile([R, 128], F32R)
            nc.sync.dma_start(out=wqk5_sb, in_=wqk5_d[:])
            wv5_sb = singles.tile([R, R], F32R)
            nc.sync.dma_start(out=wv5_sb, in_=wv5_d[:])
            ao_sb = singles.tile([R, R], F32R)
            nc.sync.dma_start(out=ao_sb, in_=ao_d[:])
            ao1_sb = singles.tile([R, R], F32R)
            nc.sync.dma_start(out=ao1_sb, in_=ao1_d[:])
            ao5_sb = singles.tile([R, H], F32R)
            nc.sync.dma_start(out=ao5_sb, in_=ao5_d[:])
            g_sb = singles.tile([H, 5], F32R)
            nc.sync.dma_start(out=g_sb, in_=g_d[:])


            def attn_tail(pqk, pv, lay, ab, inv_aps):
                """pqk: [128, 2, NA] psum (pair) of raw [Aq m; Ak m];
                pv: [R, 2, NA] psum. Returns o_sb [R+1, 2, NA] f32r."""
                tq = work.tile([R, 2, NA], F32R, tag="tqw")
                nc.scalar.activation(tq, pqk[0:R, :, :], AF.Tanh, scale=0.5)
                tk = work.tile([R, 2, NA], F32R, tag="tkw")
                nc.scalar.activation(tk, pqk[R:128, :, :], AF.Tanh, scale=0.5)
                tv = work.tile([R, 2, NA], F32, tag="tvw")
                nc.scalar.activation(tv, pv, AF.Tanh, scale=0.5)
                v = work.tile([128, 2, 2, R + 1], F32R, tag="vw")
                nc.scalar.activation(v[:, :, :, R:R + 1], ones4, AF.Copy)
                pvt = pb.tile([128, 2, 2, R], F32, tag="pbx")
                for j in range(2):
                    for mc in range(2):
                        nc.tensor.transpose(pvt[:, mc, j, :],
                                            tv[:, j, mc * 128:(mc + 1) * 128],
                                            ident[0:R, 0:R])
                nc.vector.tensor_scalar_add(v[:, :, :, 0:R], pvt, 1.0)
                e_sb = work.tile([128, 2, 2, NA], F32R, tag="ew")
                for j in range(2):
                    ps = pa.tile([128, 2, NA], F32, tag="pax")
                    pck = pb.tile([128, 2, 2], F32, tag="pbx")
                    for mc in range(2):
                        nc.tensor.matmul(ps[:, mc, :],
                                         tk[:, j, mc * 128:(mc + 1) * 128],
                                         tq[:, j, :])
                        nc.tensor.matmul(pck[:, mc, :],
                                         tk[:, j, mc * 128:(mc + 1) * 128],
                                         ones_tall[0:R, 0:2])
                    for mc in range(2):
                        bias_sb = small.tile([128, 1], F32, tag="bias")
                        nc.vector.tensor_scalar(bias_sb, pck[:, mc, 0:1],
                                                inv_aps[j], None, ALU.mult)
                        nc.scalar.activation(e_sb[:, j, mc, :], ps[:, mc, :],
                                             AF.Exp, bias=bias_sb,
                                             scale=inv_aps[j])
                po = pb.tile([R + 1, 2, NA], F32, tag="pbx")
                for j in range(2):
                    for mc in range(2):
                        nc.tensor.matmul(po[:, j, :], v[:, mc, j, :],
                                         e_sb[:, j, mc, :],
                                         start=(mc == 0), stop=(mc == 1))
                o_sb = work.tile([R + 1, 2, NA], F32R, tag="ow")
                nc.vector.tensor_copy(o_sb, po)
                return o_sb

            def proj_norm(o_sb, aoT_l, lay):
                """silu((Ao@o)/Z) via y'=(0.25Ao@P)*(1/Z); m = y'*(1+tanh y')"""
                pm = pb.tile([R, 2, NA], F32, tag="pbx")
                nc.tensor.matmul(pm, aoT_l, o_sb[0:R, :, :])
                pzr = pa.tile([R, 2, NA], F32, tag="pax")
                nc.tensor.matmul(pzr, ones_tall[R:R + 1, 0:R],
                                 o_sb[R:R + 1, :, :])
                zrec = work.tile([R, 2, NA], F32, tag="zrw")
                nc.vector.reciprocal(zrec, pzr)
                mm = work.tile([R, 2, NA], F32, tag="mmw")
                nc.vector.tensor_mul(mm, pm, zrec)
                t_sb = work.tile([R, 2, NA], F32, tag="tw")
                nc.scalar.activation(t_sb, mm, AF.Tanh)
                m_sb = work.tile([R, 2, NA], F32R, tag="mw")
                nc.vector.scalar_tensor_tensor(m_sb, t_sb, 1.0, mm,
                                               ALU.add, ALU.mult)
                return m_sb

            def stage_a(pr):
                x_sb = xpool.tile([128, KC, 2, NA], F32R, tag="x")
                nc.sync.dma_start(out=x_sb, in_=x_d[pr])
                pqk = pfp.tile([128, 2, NA], F32, tag="pfq")
                for k in range(KC):
                    nc.tensor.matmul(pqk, wqk_sb[:, k, :], x_sb[:, k, :, :],
                                     start=(k == 0), stop=(k == KC - 1))
                pv = pfp.tile([R, 2, NA], F32, tag="pfv")
                for k in range(KC):
                    nc.tensor.matmul(pv, wv_sb[:, k, :], x_sb[:, k, :, :],
                                     start=(k == 0), stop=(k == KC - 1))
                return x_sb, pqk, pv

            def stage_b(pr, x_sb, pqk, pv):
                ab = pr % 2
                b0 = pr * 2
                inv_aps = [invb4[:, b0:b0 + 1], invb4[:, b0 + 1:b0 + 2]]
                o1 = attn_tail(pqk, pv, 0, ab, inv_aps)
                m1 = proj_norm(o1, ao_sb, 0)

                pqk2 = pa.tile([128, 2, NA], F32, tag="pax")
                nc.tensor.matmul(pqk2, wqk1_sb, m1)
                pv2 = pb.tile([R, 2, NA], F32, tag="pbx")
                nc.tensor.matmul(pv2, wv1_sb, m1)
                o2 = attn_tail(pqk2, pv2, 1, ab, inv_aps)
                m2 = proj_norm(o2, ao1_sb, 1)

                pqk3 = pa.tile([128, 2, NA], F32, tag="pax")
                nc.tensor.matmul(pqk3, wqk5_sb, m2)
                pv3 = pb.tile([R, 2, NA], F32, tag="pbx")
                nc.tensor.matmul(pv3, wv5_sb, m2)
                o3 = attn_tail(pqk3, pv3, 2, ab, inv_aps)
                m4 = proj_norm(o3, ao5_sb, 2)

                # ---- epilogue (paired where possible) ----
                m4sq = work.tile([H, 2, NA], F32R, tag="m4sq")
                nc.gpsimd.tensor_mul(m4sq, m4, m4)
                pg = pb.tile([5, 2, NA], F32, tag="pbx")
                nc.tensor.matmul(pg, g_sb, m4sq)
                sg = small.tile([5, 2, NA], F32, tag="sg")
                nc.scalar.activation(sg, pg, AF.Copy)
                for j in range(2):
                    b = b0 + j
                    se = small.tile([4, 4 * 128], F32, tag=f"se{j}")
                    sg_pairs = sg[0:4, j, :].rearrange("p (t e) -> p e t", e=2)
                    for c in range(4):
                        off = 1 if c >= 2 else 0
                        nc.sync.dma_start(out=se[c:c + 1, :],
                                          in_=sg_pairs[:, off:off + 1, :])
                    x4a = x_sb[0:4, 0, j, 0:128]
                    x4b = x_sb[0:4, 0, j, 128:256]
                    pp = small.tile([4, 4 * 128], F32, tag=f"pp{j}")
                    nc.gpsimd.tensor_mul(pp[:, 0:128], x4a, x4a)
                    nc.gpsimd.tensor_mul(pp[:, 128:256], x4a, x4b)
                    nc.gpsimd.tensor_copy(pp[:, 256:384], pp[:, 128:256])
                    nc.gpsimd.tensor_mul(pp[:, 384:512], x4b, x4b)
                    fin = small.tile([5, 1], F32, tag=f"fin{j}")
                    nc.vector.reduce_sum(fin, sg[:, j, :],
                                         axis=mybir.AxisListType.X)
                    scr = small.tile([4, 4 * 128], F32, tag=f"scr{j}")
                    nc.vector.tensor_mul(scr, pp, se)
                    nc.vector.reduce_sum(fin[0:4, :], scr,
                                         axis=mybir.AxisListType.X)
                    pfin = pb.tile([1, 1], F32, tag="pbx")
                    nc.tensor.matmul(pfin, fin, ones5)
                    nc.vector.tensor_copy(res_sb[:, b:b + 1], pfin)

            state = stage_a(0)
            for pr in range(NP):
                nxt = stage_a(pr + 1) if pr + 1 < NP else None
                stage_b(pr, *state)
                state = nxt

            nc.sync.dma_start(out=out_d[:], in_=res_sb)

    nc.finalize()
    return nc


def host_prep(inputs):
    """Split full inputs into 8 per-core input maps."""
    x = np.ascontiguousarray(inputs["x"], dtype=np.float32)
    L = np.asarray(inputs["L"], dtype=np.float32)
    w = {
        "wqk": np.ascontiguousarray(np.concatenate([inputs["Aq"].T, inputs["Ak"].T], 1), np.float32),
        "wv": np.ascontiguousarray(inputs["Av"].T, np.float32),
        "wqk1": np.ascontiguousarray(np.concatenate([inputs["Aq1"].T, inputs["Ak1"].T], 1), np.float32),
        "wv1": np.ascontiguousarray(inputs["Av1"].T, np.float32),
        "wqk5": np.ascontiguousarray(np.concatenate([inputs["Aq5"].T, inputs["Ak5"].T], 1), np.float32),
        "wv5": np.ascontiguousarray(inputs["Av5"].T, np.float32),
        "aoT": np.ascontiguousarray(0.25 * inputs["Ao"].T, np.float32),
        "ao1T": np.ascontiguousarray(0.25 * inputs["Ao1"].T, np.float32),
        "ao5T": np.ascontiguousarray(0.25 * inputs["Ao5"].T, np.float32),
    }
    G = np.zeros((H, 5), np.float32)
    for row in range(H):
        g = row // 8
        G[row, g if g < 4 else 4] = 1.0
    w["gmat"] = G
    in_maps = []
    for core in range(NCORES):
        sl = slice(core * BL, (core + 1) * BL)
        m = dict(w)
        xc = x[sl].reshape(NP, 2, KC, 128, NA).transpose(0, 3, 2, 1, 4)
        m["x"] = np.ascontiguousarray(xc)
        m["l0"] = np.ascontiguousarray(L[sl, 0, :])
        in_maps.append(m)
    return in_maps


def kernel_run(inputs, trace=False):
    if "nc" not in _cache:
        _cache["nc"] = build_bass()
    nc = _cache["nc"]
    in_maps = host_prep(inputs)
    res = run_bass_kernel_spmd(nc, in_maps, core_ids=list(range(NCORES)),
                               trace=trace)
    outs = [res.results[c]["out"].reshape(BL, 1) for c in range(NCORES)]
    full = np.concatenate(outs, 0).astype(np.float32)
    return full, res.exec_time_ns


def kernel(**inputs):
    out, _ = kernel_run(inputs, trace=False)
    return out


# revision 33
# speedup vs baseline: 1.1845x; 1.1845x over previous
"""Trainium2 Bass kernel for nn_Att_H_47571057771075.

Data-parallel over batch: 64 batches -> 8 cores x 8 batches; on each core the
8 batches are processed as 4 pairs so every elementwise/matmul instruction
covers two batches (amortizes per-instruction fixed costs).

Performance design:
- float32r end-to-end for every matmul operand (1 cycle/row on the PE for
  moving dim >= 256, vs 4 for plain fp32).
- One ACT table set for the whole kernel (exp_and_others): sigmoid/silu come
  from tanh, softmax from exp; only the one-time numNeighbors sqrt uses
  another set.
- Q and K share one PE pass (stationary [Wq^T|Wk^T]); scores are computed
  transposed so exp runs straight off PSUM and A@V needs no transpose of A.
- sigmoid(x) = 0.5(1+tanh(x/2)): the +1 rank-1 terms of Q,K fold into an
  extra all-ones column of the scores matmul (per-key bias into exp) and are
  dropped for the per-query terms (softmax-invariant). Constant 0.5/0.25
  factors fold into host-side weights / the exp scale.
- Softmax denominator Z comes free from an all-ones stationary column in the
  A@V matmul; normalization is deferred through the linear output projection.
- Ones-columns live in persistent double-buffered tiles, written once.
"""
import os
import sys

for p in ("/opt/trn_rl_repo",):
    if p not in sys.path:
        sys.path.insert(0, p)

import numpy as np

import concourse.bacc as bacc
import concourse.tile as tile
from concourse import mybir
from concourse.masks import make_identity
from concourse.bass_utils import run_bass_kernel_spmd

F32 = mybir.dt.float32
F32R = mybir.dt.float32r
AF = mybir.ActivationFunctionType
ALU = mybir.AluOpType

B, D, NA, R, H = 64, 2048, 256, 64, 64
NCORES = 8
BL = B // NCORES          # batches per core
NP = BL // 2              # batch pairs per core
KC = D // 128             # k-chunks for layer 1 (16)

_cache = {}


def build_bass():
    nc = bacc.Bacc("TRN2", target_bir_lowering=False, debug=False)

    x_d = nc.dram_tensor("x", [NP, 128, KC, 2, NA], F32R, kind="ExternalInput")
    l_d = nc.dram_tensor("l0", [BL, NA], F32, kind="ExternalInput")
    wqk_d = nc.dram_tensor("wqk", [D, 128], F32R, kind="ExternalInput")
    wv_d = nc.dram_tensor("wv", [D, R], F32R, kind="ExternalInput")
    wqk1_d = nc.dram_tensor("wqk1", [R, 128], F32R, kind="ExternalInput")
    wv1_d = nc.dram_tensor("wv1", [R, R], F32R, kind="ExternalInput")
    wqk5_d = nc.dram_tensor("wqk5", [R, 128], F32R, kind="ExternalInput")
    wv5_d = nc.dram_tensor("wv5", [R, R], F32R, kind="ExternalInput")
    ao_d = nc.dram_tensor("aoT", [R, R], F32R, kind="ExternalInput")
    ao1_d = nc.dram_tensor("ao1T", [R, R], F32R, kind="ExternalInput")
    ao5_d = nc.dram_tensor("ao5T", [R, H], F32R, kind="ExternalInput")
    g_d = nc.dram_tensor("gmat", [H, 5], F32R, kind="ExternalInput")
    out_d = nc.dram_tensor("out", [1, BL], F32, kind="ExternalOutput")

    with tile.TileContext(nc) as tc:
        with (
            tc.tile_pool(name="singles", bufs=1) as singles,
            tc.tile_pool(name="xpool", bufs=2) as xpool,
            tc.tile_pool(name="work", bufs=3) as work,
            tc.tile_pool(name="small", bufs=2) as small,
            tc.tile_pool(name="pa", bufs=3, space="PSUM") as pa,
            tc.tile_pool(name="pb", bufs=3, space="PSUM") as pb,
            tc.tile_pool(name="pf", bufs=2, space="PSUM") as pfp,
        ):
            # ---- per-batch 0.25/sqrt(numN) first (one sqrt-table load) ----
            l_sb = small.tile([BL, NA], F32, tag="lsb")
            nc.sync.dma_start(out=l_sb, in_=l_d[:])
            ind = small.tile([BL, NA], F32, tag="ind")
            nc.vector.tensor_single_scalar(ind, l_sb, 1.0, ALU.is_ge)
            s8 = small.tile([BL, 1], F32, tag="s8")
            nc.vector.reduce_sum(s8, ind, axis=mybir.AxisListType.X)
            c16 = small.tile([BL, 1], F32, tag="c16")
            nc.vector.memset(c16, 16.0)
            nc.scalar.activation(s8, s8, AF.Sqrt, bias=c16, scale=16.0)
            inv8 = small.tile([BL, 1], F32, tag="inv8")
            nc.vector.reciprocal(inv8, s8)

            ident = singles.tile([128, 128], F32)
            make_identity(nc, ident[:])
            ones_sb = singles.tile([1, 128], F32)
            nc.vector.memset(ones_sb, 1.0)
            ones_f32 = singles.tile([128, R], F32)
            nc.vector.memset(ones_f32, 1.0)
            ones_tall = singles.tile([128, R], F32R)
            nc.scalar.activation(ones_tall, ones_f32, AF.Copy)
            onesq = singles.tile([R, 2, 2], F32)
            nc.vector.memset(onesq, 1.0)
            ones4 = singles.tile([128, 2, 2, 1], F32)
            nc.vector.memset(ones4, 1.0)
            ones5 = singles.tile([5, 1], F32)
            nc.vector.memset(ones5, 1.0)
            res_sb = singles.tile([1, BL], F32)

            pt = pfp.tile([1, BL], F32, tag="pff")
            nc.tensor.transpose(pt, inv8, ident[0:BL, 0:BL])
            invrow = small.tile([1, BL], F32, tag="invrow")
            nc.vector.tensor_copy(invrow, pt)
            pib = pfp.tile([128, BL], F32, tag="pff")
            nc.tensor.matmul(pib, ones_sb, invrow)
            invb4 = singles.tile([128, BL], F32)
            nc.vector.tensor_copy(invb4, pib)

            # ---- weights ----
            wqk_sb = singles.tile([128, KC, 128], F32R)
            nc.sync.dma_start(out=wqk_sb, in_=wqk_d[:].rearrange("(c p) m -> p c m", p=128))
            wv_sb = singles.tile([128, KC, R], F32R)
            nc.sync.dma_start(out=wv_sb, in_=wv_d[:].rearrange("(c p) m -> p c m", p=128))
            wqk1_sb = singles.tile([R, 128], F32R)
            nc.sync.dma_start(out=wqk1_sb, in_=wqk1_d[:])
            wv1_sb = singles.tile([R, R], F32R)
            nc.sync.dma_start(out=wv1_sb, in_=wv1_d[:])
            wqk5_sb = singles.tile([R, 128], F32R)
            nc.sync.dma_start(out=wqk5_sb, in_=wqk5_d[:])
            wv5_sb = singles.tile([R, R], F32R)
            nc.sync.dma_start(out=wv5_sb, in_=wv5_d[:])
            ao_sb = singles.tile([R, R], F32R)
            nc.sync.dma_start(out=ao_sb, in_=ao_d[:])
            ao1_sb = singles.tile([R, R], F32R)
            nc.sync.dma_start(out=ao1_sb, in_=ao1_d[:])
            ao5_sb = singles.tile([R, H], F32R)
            nc.sync.dma_start(out=ao5_sb, in_=ao5_d[:])
            g_sb = singles.tile([H, 5], F32R)
            nc.sync.dma_start(out=g_sb, in_=g_d[:])


            def attn_tail(pqk, pv, lay, ab, inv_aps):
                """pqk: [128, 2, NA] psum (pair) of raw [Aq m; Ak m];
                pv: [R, 2, NA] psum. Returns o_sb [R+1, 2, NA] f32r."""
                tq = work.tile([R, 2, NA + 2], F32R, tag="tqw")
                nc.scalar.activation(tq[:, :, 0:NA], pqk[0:R, :, :], AF.Tanh,
                                     scale=0.5)
                nc.scalar.activation(tq[:, :, NA:NA + 2], onesq, AF.Copy)
                tk = work.tile([R, 2, NA], F32R, tag="tkw")
                nc.scalar.activation(tk, pqk[R:128, :, :], AF.Tanh, scale=0.5)
                tv = work.tile([R, 2, NA], F32, tag="tvw")
                nc.scalar.activation(tv, pv, AF.Tanh, scale=0.5)
                v = work.tile([128, 2, 2, R + 1], F32R, tag="vw")
                nc.scalar.activation(v[:, :, :, R:R + 1], ones4, AF.Copy)
                pvt = pb.tile([128, 2, 2, R], F32, tag="pbx")
                for j in range(2):
                    for mc in range(2):
                        nc.tensor.transpose(pvt[:, mc, j, :],
                                            tv[:, j, mc * 128:(mc + 1) * 128],
                                            ident[0:R, 0:R])
                nc.vector.tensor_scalar_add(v[:, :, :, 0:R], pvt, 1.0)
                e_sb = work.tile([128, 2, 2, NA], F32R, tag="ew")
                for j in range(2):
                    for mc in range(2):
                        ps = pa.tile([128, NA + 2], F32, tag="pax")
                        nc.tensor.matmul(ps, tk[:, j, mc * 128:(mc + 1) * 128],
                                         tq[:, j, :])
                        bias_sb = small.tile([128, 1], F32, tag="bias")
                        nc.vector.tensor_scalar(bias_sb, ps[:, NA:NA + 1],
                                                inv_aps[j], None, ALU.mult)
                        nc.scalar.activation(e_sb[:, j, mc, :], ps[:, 0:NA],
                                             AF.Exp, bias=bias_sb,
                                             scale=inv_aps[j])
                po = pb.tile([R + 1, 2, NA], F32, tag="pbx")
                for j in range(2):
                    for mc in range(2):
                        nc.tensor.matmul(po[:, j, :], v[:, mc, j, :],
                                         e_sb[:, j, mc, :],
                                         start=(mc == 0), stop=(mc == 1))
                o_sb = work.tile([R + 1, 2, NA], F32R, tag="ow")
                nc.vector.tensor_copy(o_sb, po)
                return o_sb

            def proj_norm(o_sb, aoT_l, lay):
                """silu((Ao@o)/Z) via y'=(0.25Ao@P)*(1/Z); m = y'*(1+tanh y')"""
                pm = pb.tile([R, 2, NA], F32, tag="pbx")
                nc.tensor.matmul(pm, aoT_l, o_sb[0:R, :, :])
                pzr = pa.tile([R, 2, NA], F32, tag="pax")
                nc.tensor.matmul(pzr, ones_tall[R:R + 1, 0:R],
                                 o_sb[R:R + 1, :, :])
                zrec = work.tile([R, 2, NA], F32, tag="zrw")
                nc.vector.reciprocal(zrec, pzr)
                mm = work.tile([R, 2, NA], F32, tag="mmw")
                nc.vector.tensor_mul(mm, pm, zrec)
                t_sb = work.tile([R, 2, NA], F32, tag="tw")
                nc.scalar.activation(t_sb, mm, AF.Tanh)
                m_sb = work.tile([R, 2, NA], F32R, tag="mw")
                nc.vector.scalar_tensor_tensor(m_sb, t_sb, 1.0, mm,
                                               ALU.add, ALU.mult)
                return m_sb

            def stage_a(pr):
                x_sb = xpool.tile([128, KC, 2, NA], F32R, tag="x")
                nc.sync.dma_start(out=x_sb, in_=x_d[pr])
                pqk = pa.tile([128, 2, NA], F32, tag="pax")
                for k in range(KC):
                    nc.tensor.matmul(pqk, wqk_sb[:, k, :], x_sb[:, k, :, :],
                                     start=(k == 0), stop=(k == KC - 1))
                pv = pb.tile([R, 2, NA], F32, tag="pbx")
                for k in range(KC):
                    nc.tensor.matmul(pv, wv_sb[:, k, :], x_sb[:, k, :, :],
                                     start=(k == 0), stop=(k == KC - 1))
                return x_sb, pqk, pv

            def stage_b(pr, x_sb, pqk, pv):
                ab = pr % 2
                b0 = pr * 2
                inv_aps = [invb4[:, b0:b0 + 1], invb4[:, b0 + 1:b0 + 2]]
                o1 = attn_tail(pqk, pv, 0, ab, inv_aps)
                m1 = proj_norm(o1, ao_sb, 0)

                pqk2 = pa.tile([128, 2, NA], F32, tag="pax")
                nc.tensor.matmul(pqk2, wqk1_sb, m1)
                pv2 = pb.tile([R, 2, NA], F32, tag="pbx")
                nc.tensor.matmul(pv2, wv1_sb, m1)
                o2 = attn_tail(pqk2, pv2, 1, ab, inv_aps)
                m2 = proj_norm(o2, ao1_sb, 1)

                pqk3 = pa.tile([128, 2, NA], F32, tag="pax")
                nc.tensor.matmul(pqk3, wqk5_sb, m2)
                pv3 = pb.tile([R, 2, NA], F32, tag="pbx")
                nc.tensor.matmul(pv3, wv5_sb, m2)
                o3 = attn_tail(pqk3, pv3, 2, ab, inv_aps)
                m4 = proj_norm(o3, ao5_sb, 2)

                # ---- epilogue (paired where possible) ----
                m4sq = work.tile([H, 2, NA], F32R, tag="m4sq")
                nc.gpsimd.tensor_mul(m4sq, m4, m4)
                pg = pfp.tile([5, 2, NA], F32, tag="pff")
                nc.tensor.matmul(pg, g_sb, m4sq)
                sg = small.tile([5, 2, NA], F32, tag="sg")
                nc.scalar.activation(sg, pg, AF.Copy)
                for j in range(2):
                    b = b0 + j
                    se = small.tile([4, 4 * 128], F32, tag=f"se{j}")
                    sg_pairs = sg[0:4, j, :].rearrange("p (t e) -> p e t", e=2)
                    for c in range(4):
                        off = 1 if c >= 2 else 0
                        nc.sync.dma_start(out=se[c:c + 1, :],
                                          in_=sg_pairs[:, off:off + 1, :])
                    x4a = x_sb[0:4, 0, j, 0:128]
                    x4b = x_sb[0:4, 0, j, 128:256]
                    pp = small.tile([4, 4 * 128], F32, tag=f"pp{j}")
                    nc.gpsimd.tensor_mul(pp[:, 0:128], x4a, x4a)
                    nc.gpsimd.tensor_mul(pp[:, 128:256], x4a, x4b)
                    nc.gpsimd.tensor_copy(pp[:, 256:384], pp[:, 128:256])
                    nc.gpsimd.tensor_mul(pp[:, 384:512], x4b, x4b)
                    fin = small.tile([5, 1], F32, tag=f"fin{j}")
                    nc.vector.reduce_sum(fin, sg[:, j, :],
                                         axis=mybir.AxisListType.X)
                    scr = small.tile([4, 4 * 128], F32, tag=f"scr{j}")
                    nc.vector.tensor_mul(scr, pp, se)
                    nc.vector.reduce_sum(fin[0:4, :], scr,
                                         axis=mybir.AxisListType.X)
                    pfin = pfp.tile([1, 1], F32, tag="pff")
                    nc.tensor.matmul(pfin, fin, ones5)
                    nc.vector.tensor_copy(res_sb[:, b:b + 1], pfin)

            for pr in range(NP):
                stage_b(pr, *stage_a(pr))

            nc.sync.dma_start(out=out_d[:], in_=res_sb)

    nc.finalize()
    return nc


def host_prep(inputs):
    """Split full inputs into 8 per-core input maps."""
    x = np.ascontiguousarray(inputs["x"], dtype=np.float32)
    L = np.asarray(inputs["L"], dtype=np.float32)
    w = {
        "wqk": np.ascontiguousarray(np.concatenate([inputs["Aq"].T, inputs["Ak"].T], 1), np.float32),
        "wv": np.ascontiguousarray(inputs["Av"].T, np.float32),
        "wqk1": np.ascontiguousarray(np.concatenate([inputs["Aq1"].T, inputs["Ak1"].T], 1), np.float32),
        "wv1": np.ascontiguousarray(inputs["Av1"].T, np.float32),
        "wqk5": np.ascontiguousarray(np.concatenate([inputs["Aq5"].T, inputs["Ak5"].T], 1), np.float32),
        "wv5": np.ascontiguousarray(inputs["Av5"].T, np.float32),
        "aoT": np.ascontiguousarray(0.25 * inputs["Ao"].T, np.float32),
        "ao1T": np.ascontiguousarray(0.25 * inputs["Ao1"].T, np.float32),
        "ao5T": np.ascontiguousarray(0.25 * inputs["Ao5"].T, np.float32),
    }
    G = np.zeros((H, 5), np.float32)
    for row in range(H):
        g = row // 8
        G[row, g if g < 4 else 4] = 1.0
    w["gmat"] = G
    in_maps = []
    for core in range(NCORES):
        sl = slice(core * BL, (core + 1) * BL)
        m = dict(w)
        xc = x[sl].reshape(NP, 2, KC, 128, NA).transpose(0, 3, 2, 1, 4)
        m["x"] = np.ascontiguousarray(xc)
        m["l0"] = np.ascontiguousarray(L[sl, 0, :])
        in_maps.append(m)
    return in_maps


def kernel_run(inputs, trace=False):
    if "nc" not in _cache:
        _cache["nc"] = build_bass()
    nc = _cache["nc"]
    in_maps = host_prep(inputs)
    res = run_bass_kernel_spmd(nc, in_maps, core_ids=list(range(NCORES)),
                               trace=trace)
    outs = [res.results[c]["out"].reshape(BL, 1) for c in range(NCORES)]
    full = np.concatenate(outs, 0).astype(np.float32)
    return full, res.exec_time_ns


def kernel(**inputs):
    out, _ = kernel_run(inputs, trace=False)
    return out


# revision 40
# speedup vs baseline: 1.2120x; 1.0233x over previous
"""Trainium2 Bass kernel for nn_Att_H_47571057771075.

Data-parallel over batch: 64 batches -> 8 cores x 8 batches; on each core the
8 batches are processed as 4 pairs so every elementwise/matmul instruction
covers two batches (amortizes per-instruction fixed costs).

Performance design:
- float32r end-to-end for every matmul operand (1 cycle/row on the PE for
  moving dim >= 256, vs 4 for plain fp32).
- One ACT table set for the whole kernel (exp_and_others): sigmoid/silu come
  from tanh, softmax from exp; only the one-time numNeighbors sqrt uses
  another set.
- Q and K share one PE pass (stationary [Wq^T|Wk^T]); scores are computed
  transposed so exp runs straight off PSUM and A@V needs no transpose of A.
- sigmoid(x) = 0.5(1+tanh(x/2)): the +1 rank-1 terms of Q,K fold into an
  extra all-ones column of the scores matmul (per-key bias into exp) and are
  dropped for the per-query terms (softmax-invariant). Constant 0.5/0.25
  factors fold into host-side weights / the exp scale.
- Softmax denominator Z comes free from an all-ones stationary column in the
  A@V matmul; normalization is deferred through the linear output projection.
- Ones-columns live in persistent double-buffered tiles, written once.
"""
import os
import sys

for p in ("/opt/trn_rl_repo",):
    if p not in sys.path:
        sys.path.insert(0, p)

import numpy as np

import concourse.bacc as bacc
import concourse.tile as tile
from concourse import mybir
from concourse.masks import make_identity
from concourse.bass_utils import run_bass_kernel_spmd

F32 = mybir.dt.float32
F32R = mybir.dt.float32r
AF = mybir.ActivationFunctionType
ALU = mybir.AluOpType

B, D, NA, R, H = 64, 2048, 256, 64, 64
NCORES = 8
BL = B // NCORES          # batches per core
NP = BL // 2              # batch pairs per core
KC = D // 128             # k-chunks for layer 1 (16)

_cache = {}


def build_bass():
    nc = bacc.Bacc("TRN2", target_bir_lowering=False, debug=False)

    x_d = nc.dram_tensor("x", [NP, 128, KC, 2, NA], F32R, kind="ExternalInput")
    l_d = nc.dram_tensor("l0", [BL, NA], F32, kind="ExternalInput")
    wqk_d = nc.dram_tensor("wqk", [D, 128], F32R, kind="ExternalInput")
    wv_d = nc.dram_tensor("wv", [D, R], F32R, kind="ExternalInput")
    wqk1_d = nc.dram_tensor("wqk1", [R, 128], F32R, kind="ExternalInput")
    wv1_d = nc.dram_tensor("wv1", [R, R], F32R, kind="ExternalInput")
    wqk5_d = nc.dram_tensor("wqk5", [R, 128], F32R, kind="ExternalInput")
    wv5_d = nc.dram_tensor("wv5", [R, R], F32R, kind="ExternalInput")
    ao_d = nc.dram_tensor("aoT", [R, R], F32R, kind="ExternalInput")
    ao1_d = nc.dram_tensor("ao1T", [R, R], F32R, kind="ExternalInput")
    ao5_d = nc.dram_tensor("ao5T", [R, H], F32R, kind="ExternalInput")
    g_d = nc.dram_tensor("gmat", [H, 5], F32R, kind="ExternalInput")
    out_d = nc.dram_tensor("out", [1, BL], F32, kind="ExternalOutput")

    with tile.TileContext(nc) as tc:
        with (
            tc.tile_pool(name="singles", bufs=1) as singles,
            tc.tile_pool(name="xpool", bufs=2) as xpool,
            tc.tile_pool(name="work", bufs=3) as work,
            tc.tile_pool(name="small", bufs=2) as small,
            tc.tile_pool(name="pa", bufs=3, space="PSUM") as pa,
            tc.tile_pool(name="pb", bufs=3, space="PSUM") as pb,
            tc.tile_pool(name="pf", bufs=2, space="PSUM") as pfp,
        ):
            # ---- per-batch 0.25/sqrt(numN) first (one sqrt-table load) ----
            l_sb = small.tile([BL, NA], F32, tag="lsb")
            nc.sync.dma_start(out=l_sb, in_=l_d[:])
            ind = small.tile([BL, NA], F32, tag="ind")
            nc.vector.tensor_single_scalar(ind, l_sb, 1.0, ALU.is_ge)
            s8 = small.tile([BL, 1], F32, tag="s8")
            nc.vector.reduce_sum(s8, ind, axis=mybir.AxisListType.X)
            c16 = small.tile([BL, 1], F32, tag="c16")
            nc.vector.memset(c16, 16.0)
            nc.scalar.activation(s8, s8, AF.Sqrt, bias=c16, scale=16.0)
            inv8 = small.tile([BL, 1], F32, tag="inv8")
            nc.vector.reciprocal(inv8, s8)

            ident = singles.tile([128, 128], F32)
            make_identity(nc, ident[:])
            ones_sb = singles.tile([1, 128], F32)
            nc.vector.memset(ones_sb, 1.0)
            ones_f32 = singles.tile([128, R], F32)
            nc.vector.memset(ones_f32, 1.0)
            ones_tall = singles.tile([128, R], F32R)
            nc.scalar.activation(ones_tall, ones_f32, AF.Copy)
            onesq = singles.tile([R, 2, 2], F32)
            nc.vector.memset(onesq, 1.0)
            ones4 = singles.tile([128, 2, 2, 1], F32)
            nc.vector.memset(ones4, 1.0)
            ones5 = singles.tile([5, 1], F32)
            nc.vector.memset(ones5, 1.0)
            res_sb = singles.tile([1, BL], F32)

            pt = pfp.tile([1, BL], F32, tag="pff")
            nc.tensor.transpose(pt, inv8, ident[0:BL, 0:BL])
            invrow = small.tile([1, BL], F32, tag="invrow")
            nc.vector.tensor_copy(invrow, pt)
            pib = pfp.tile([128, BL], F32, tag="pff")
            nc.tensor.matmul(pib, ones_sb, invrow)
            invb4 = singles.tile([128, BL], F32)
            nc.vector.tensor_copy(invb4, pib)

            # ---- weights ----
            wqk_sb = singles.tile([128, KC, 128], F32R)
            nc.sync.dma_start(out=wqk_sb, in_=wqk_d[:].rearrange("(c p) m -> p c m", p=128))
            wv_sb = singles.tile([128, KC, R], F32R)
            nc.sync.dma_start(out=wv_sb, in_=wv_d[:].rearrange("(c p) m -> p c m", p=128))
            wqk1_sb = singles.tile([R, 128], F32R)
            nc.sync.dma_start(out=wqk1_sb, in_=wqk1_d[:])
            wv1_sb = singles.tile([R, R], F32R)
            nc.sync.dma_start(out=wv1_sb, in_=wv1_d[:])
            wqk5_sb = singles.tile([R, 128], F32R)
            nc.sync.dma_start(out=wqk5_sb, in_=wqk5_d[:])
            wv5_sb = singles.tile([R, R], F32R)
            nc.sync.dma_start(out=wv5_sb, in_=wv5_d[:])
            ao_sb = singles.tile([R, R], F32R)
            nc.sync.dma_start(out=ao_sb, in_=ao_d[:])
            ao1_sb = singles.tile([R, R], F32R)
            nc.sync.dma_start(out=ao1_sb, in_=ao1_d[:])
            ao5_sb = singles.tile([R, H], F32R)
            nc.sync.dma_start(out=ao5_sb, in_=ao5_d[:])
            g_sb = singles.tile([H, 5], F32R)
            nc.sync.dma_start(out=g_sb, in_=g_d[:])


            def attn_tail(pqk, pv, lay, ab, inv_aps):
                """pqk: [128, 2, NA] psum (pair) of raw [Aq m; Ak m];
                pv: [R, 2, NA] psum. Returns o_sb [R+1, 2, NA] f32r."""
                tq = work.tile([R, 2, NA + 2], F32R, tag="tqw")
                nc.scalar.activation(tq[:, :, 0:NA], pqk[0:R, :, :], AF.Tanh,
                                     scale=0.5)
                nc.scalar.activation(tq[:, :, NA:NA + 2], onesq, AF.Copy)
                tk = work.tile([R, 2, NA], F32R, tag="tkw")
                nc.scalar.activation(tk, pqk[R:128, :, :], AF.Tanh, scale=0.5)
                tv = work.tile([R, 2, NA], F32, tag="tvw")
                nc.scalar.activation(tv, pv, AF.Tanh, scale=0.5)
                v = work.tile([128, 2, 2, R + 1], F32R, tag="vw")
                nc.scalar.activation(v[:, :, :, R:R + 1], ones4, AF.Copy)
                pvt = pb.tile([128, 2, 2, R], F32, tag="pbx")
                for j in range(2):
                    for mc in range(2):
                        nc.tensor.transpose(pvt[:, mc, j, :],
                                            tv[:, j, mc * 128:(mc + 1) * 128],
                                            ident[0:R, 0:R])
                nc.vector.tensor_scalar_add(v[:, :, :, 0:R], pvt, 1.0)
                e_sb = work.tile([128, 2, 2, NA], F32R, tag="ew")
                for j in range(2):
                    for mc in range(2):
                        ps = pa.tile([128, NA + 2], F32, tag="pax")
                        nc.tensor.matmul(ps, tk[:, j, mc * 128:(mc + 1) * 128],
                                         tq[:, j, :])
                        bias_sb = small.tile([128, 1], F32, tag="bias")
                        nc.vector.tensor_scalar(bias_sb, ps[:, NA:NA + 1],
                                                inv_aps[j], None, ALU.mult)
                        nc.scalar.activation(e_sb[:, j, mc, :], ps[:, 0:NA],
                                             AF.Exp, bias=bias_sb,
                                             scale=inv_aps[j])
                po = pb.tile([R + 1, 2, NA], F32, tag="pbx")
                for j in range(2):
                    for mc in range(2):
                        nc.tensor.matmul(po[:, j, :], v[:, mc, j, :],
                                         e_sb[:, j, mc, :],
                                         start=(mc == 0), stop=(mc == 1))
                o_sb = work.tile([R + 1, 2, NA], F32R, tag="ow")
                nc.vector.tensor_copy(o_sb, po)
                return o_sb

            def proj_norm(o_sb, aoT_l, lay):
                """silu((Ao@o)/Z) via y'=(0.25Ao@P)*(1/Z); m = y'*(1+tanh y')"""
                if lay >= 1:
                    pm = pfp.tile([R, 2, NA], F32, tag="pff")
                else:
                    pm = pb.tile([R, 2, NA], F32, tag="pbx")
                if lay == 2:
                    pzr = pfp.tile([R, 2, NA], F32, tag="pff")
                else:
                    pzr = pa.tile([R, 2, NA], F32, tag="pax")
                nc.tensor.matmul(pm, aoT_l, o_sb[0:R, :, :])
                nc.tensor.matmul(pzr, ones_tall[R:R + 1, 0:R],
                                 o_sb[R:R + 1, :, :])
                zrec = work.tile([R, 2, NA], F32, tag="zrw")
                nc.vector.reciprocal(zrec, pzr)
                mm = work.tile([R, 2, NA], F32, tag="mmw")
                nc.vector.tensor_mul(mm, pm, zrec)
                t_sb = work.tile([R, 2, NA], F32, tag="tw")
                nc.scalar.activation(t_sb, mm, AF.Tanh)
                m_sb = work.tile([R, 2, NA], F32R, tag="mw")
                nc.vector.scalar_tensor_tensor(m_sb, t_sb, 1.0, mm,
                                               ALU.add, ALU.mult)
                return m_sb

            def stage_a(pr):
                x_sb = xpool.tile([128, KC, 2, NA], F32R, tag="x")
                nc.sync.dma_start(out=x_sb, in_=x_d[pr])
                pqk = pa.tile([128, 2, NA], F32, tag="pax")
                for k in range(KC):
                    nc.tensor.matmul(pqk, wqk_sb[:, k, :], x_sb[:, k, :, :],
                                     start=(k == 0), stop=(k == KC - 1))
                pv = pb.tile([R, 2, NA], F32, tag="pbx")
                for k in range(KC):
                    nc.tensor.matmul(pv, wv_sb[:, k, :], x_sb[:, k, :, :],
                                     start=(k == 0), stop=(k == KC - 1))
                return x_sb, pqk, pv

            def stage_b(pr, x_sb, pqk, pv):
                ab = pr % 2
                b0 = pr * 2
                inv_aps = [invb4[:, b0:b0 + 1], invb4[:, b0 + 1:b0 + 2]]
                o1 = attn_tail(pqk, pv, 0, ab, inv_aps)
                m1 = proj_norm(o1, ao_sb, 0)

                pqk2 = pa.tile([128, 2, NA], F32, tag="pax")
                nc.tensor.matmul(pqk2, wqk1_sb, m1)
                pv2 = pb.tile([R, 2, NA], F32, tag="pbx")
                nc.tensor.matmul(pv2, wv1_sb, m1)
                o2 = attn_tail(pqk2, pv2, 1, ab, inv_aps)
                m2 = proj_norm(o2, ao1_sb, 1)

                pqk3 = pa.tile([128, 2, NA], F32, tag="pax")
                nc.tensor.matmul(pqk3, wqk5_sb, m2)
                pv3 = pb.tile([R, 2, NA], F32, tag="pbx")
                nc.tensor.matmul(pv3, wv5_sb, m2)
                o3 = attn_tail(pqk3, pv3, 2, ab, inv_aps)
                m4 = proj_norm(o3, ao5_sb, 2)

                # ---- epilogue (paired where possible) ----
                m4sq = work.tile([H, 2, NA], F32R, tag="m4sq")
                nc.gpsimd.tensor_mul(m4sq, m4, m4)
                pg = pfp.tile([5, 2, NA], F32, tag="pff")
                nc.tensor.matmul(pg, g_sb, m4sq)
                sg = small.tile([5, 2, NA], F32, tag="sg")
                nc.scalar.activation(sg, pg, AF.Copy)
                for j in range(2):
                    b = b0 + j
                    se = small.tile([4, 4 * 128], F32, tag=f"se{j}")
                    sg_pairs = sg[0:4, j, :].rearrange("p (t e) -> p e t", e=2)
                    for c in range(4):
                        off = 1 if c >= 2 else 0
                        nc.sync.dma_start(out=se[c:c + 1, :],
                                          in_=sg_pairs[:, off:off + 1, :])
                    x4a = x_sb[0:4, 0, j, 0:128]
                    x4b = x_sb[0:4, 0, j, 128:256]
                    pp = small.tile([4, 4 * 128], F32, tag=f"pp{j}")
                    nc.gpsimd.tensor_mul(pp[:, 0:128], x4a, x4a)
                    nc.gpsimd.tensor_mul(pp[:, 128:256], x4a, x4b)
                    nc.gpsimd.tensor_copy(pp[:, 256:384], pp[:, 128:256])
                    nc.gpsimd.tensor_mul(pp[:, 384:512], x4b, x4b)
                    fin = small.tile([5, 1], F32, tag=f"fin{j}")
                    nc.vector.reduce_sum(fin, sg[:, j, :],
                                         axis=mybir.AxisListType.X)
                    scr = small.tile([4, 4 * 128], F32, tag=f"scr{j}")
                    nc.vector.tensor_mul(scr, pp, se)
                    nc.vector.reduce_sum(fin[0:4, :], scr,
                                         axis=mybir.AxisListType.X)
                    pfin = pfp.tile([1, 1], F32, tag="pff")
                    nc.tensor.matmul(pfin, fin, ones5)
                    nc.vector.tensor_copy(res_sb[:, b:b + 1], pfin)

            for pr in range(NP):
                stage_b(pr, *stage_a(pr))

            nc.sync.dma_start(out=out_d[:], in_=res_sb)

    nc.finalize()
    return nc


def host_prep(inputs):
    """Split full inputs into 8 per-core input maps."""
    x = np.ascontiguousarray(inputs["x"], dtype=np.float32)
    L = np.asarray(inputs["L"], dtype=np.float32)
    w = {
        "wqk": np.ascontiguousarray(np.concatenate([inputs["Aq"].T, inputs["Ak"].T], 1), np.float32),
        "wv": np.ascontiguousarray(inputs["Av"].T, np.float32),
        "wqk1": np.ascontiguousarray(np.concatenate([inputs["Aq1"].T, inputs["Ak1"].T], 1), np.float32),
        "wv1": np.ascontiguousarray(inputs["Av1"].T, np.float32),
        "wqk5": np.ascontiguousarray(np.concatenate([inputs["Aq5"].T, inputs["Ak5"].T], 1), np.float32),
        "wv5": np.ascontiguousarray(inputs["Av5"].T, np.float32),
        "aoT": np.ascontiguousarray(0.25 * inputs["Ao"].T, np.float32),
        "ao1T": np.ascontiguousarray(0.25 * inputs["Ao1"].T, np.float32),
        "ao5T": np.ascontiguousarray(0.25 * inputs["Ao5"].T, np.float32),
    }
    G = np.zeros((H, 5), np.float32)
    for row in range(H):
        g = row // 8
        G[row, g if g < 4 else 4] = 1.0
    w["gmat"] = G
    in_maps = []
    for core in range(NCORES):
        sl = slice(core * BL, (core + 1) * BL)
        m = dict(w)
        xc = x[sl].reshape(NP, 2, KC, 128, NA).transpose(0, 3, 2, 1, 4)
        m["x"] = np.ascontiguousarray(xc)
        m["l0"] = np.ascontiguousarray(L[sl, 0, :])
        in_maps.append(m)
    return in_maps


def kernel_run(inputs, trace=False):
    if "nc" not in _cache:
        _cache["nc"] = build_bass()
    nc = _cache["nc"]
    in_maps = host_prep(inputs)
    res = run_bass_kernel_spmd(nc, in_maps, core_ids=list(range(NCORES)),
                               trace=trace)
    outs = [res.results[c]["out"].reshape(BL, 1) for c in range(NCORES)]
    full = np.concatenate(outs, 0).astype(np.float32)
    return full, res.exec_time_ns


def kernel(**inputs):
    out, _ = kernel_run(inputs, trace=False)
    return out


# revision 42
# speedup vs baseline: 1.2268x; 1.0122x over previous
"""Trainium2 Bass kernel for nn_Att_H_47571057771075.

Data-parallel over batch: 64 batches -> 8 cores x 8 batches; on each core the
8 batches are processed as 4 pairs so every elementwise/matmul instruction
covers two batches (amortizes per-instruction fixed costs).

Performance design:
- float32r end-to-end for every matmul operand (1 cycle/row on the PE for
  moving dim >= 256, vs 4 for plain fp32).
- One ACT table set for the whole kernel (exp_and_others): sigmoid/silu come
  from tanh, softmax from exp; only the one-time numNeighbors sqrt uses
  another set.
- Q and K share one PE pass (stationary [Wq^T|Wk^T]); scores are computed
  transposed so exp runs straight off PSUM and A@V needs no transpose of A.
- sigmoid(x) = 0.5(1+tanh(x/2)): the +1 rank-1 terms of Q,K fold into an
  extra all-ones column of the scores matmul (per-key bias into exp) and are
  dropped for the per-query terms (softmax-invariant). Constant 0.5/0.25
  factors fold into host-side weights / the exp scale.
- Softmax denominator Z comes free from an all-ones stationary column in the
  A@V matmul; normalization is deferred through the linear output projection.
- Ones-columns live in persistent double-buffered tiles, written once.
"""
import os
import sys

for p in ("/opt/trn_rl_repo",):
    if p not in sys.path:
        sys.path.insert(0, p)

import numpy as np

import concourse.bacc as bacc
import concourse.tile as tile
from concourse import mybir
from concourse.masks import make_identity
from concourse.bass_utils import run_bass_kernel_spmd

F32 = mybir.dt.float32
F32R = mybir.dt.float32r
AF = mybir.ActivationFunctionType
ALU = mybir.AluOpType

B, D, NA, R, H = 64, 2048, 256, 64, 64
NCORES = 8
BL = B // NCORES          # batches per core
NP = BL // 2              # batch pairs per core
KC = D // 128             # k-chunks for layer 1 (16)

_cache = {}


def build_bass():
    nc = bacc.Bacc("TRN2", target_bir_lowering=False, debug=False)

    x_d = nc.dram_tensor("x", [NP, 128, KC, 2, NA], F32R, kind="ExternalInput")
    l_d = nc.dram_tensor("l0", [BL, NA], F32, kind="ExternalInput")
    wqk_d = nc.dram_tensor("wqk", [D, 128], F32R, kind="ExternalInput")
    wv_d = nc.dram_tensor("wv", [D, R], F32R, kind="ExternalInput")
    wqk1_d = nc.dram_tensor("wqk1", [R, 128], F32R, kind="ExternalInput")
    wv1_d = nc.dram_tensor("wv1", [R, R], F32R, kind="ExternalInput")
    wqk5_d = nc.dram_tensor("wqk5", [R, 128], F32R, kind="ExternalInput")
    wv5_d = nc.dram_tensor("wv5", [R, R], F32R, kind="ExternalInput")
    ao_d = nc.dram_tensor("aoT", [R, R], F32R, kind="ExternalInput")
    ao1_d = nc.dram_tensor("ao1T", [R, R], F32R, kind="ExternalInput")
    ao5_d = nc.dram_tensor("ao5T", [R, H], F32R, kind="ExternalInput")
    g_d = nc.dram_tensor("gmat", [H, 5], F32R, kind="ExternalInput")
    id_d = nc.dram_tensor("ident", [128, 128], F32, kind="ExternalInput")
    out_d = nc.dram_tensor("out", [1, BL], F32, kind="ExternalOutput")

    with tile.TileContext(nc) as tc:
        with (
            tc.tile_pool(name="singles", bufs=1) as singles,
            tc.tile_pool(name="xpool", bufs=2) as xpool,
            tc.tile_pool(name="work", bufs=3) as work,
            tc.tile_pool(name="small", bufs=2) as small,
            tc.tile_pool(name="pa", bufs=3, space="PSUM") as pa,
            tc.tile_pool(name="pb", bufs=3, space="PSUM") as pb,
            tc.tile_pool(name="pf", bufs=2, space="PSUM") as pfp,
        ):
            # ---- per-batch 0.25/sqrt(numN) first (one sqrt-table load) ----
            l_sb = small.tile([BL, NA], F32, tag="lsb")
            nc.sync.dma_start(out=l_sb, in_=l_d[:])
            ind = small.tile([BL, NA], F32, tag="ind")
            nc.vector.tensor_single_scalar(ind, l_sb, 1.0, ALU.is_ge)
            s8 = small.tile([BL, 1], F32, tag="s8")
            nc.vector.reduce_sum(s8, ind, axis=mybir.AxisListType.X)
            c16 = small.tile([BL, 1], F32, tag="c16")
            nc.vector.memset(c16, 16.0)
            nc.scalar.activation(s8, s8, AF.Sqrt, bias=c16, scale=16.0)
            inv8 = small.tile([BL, 1], F32, tag="inv8")
            nc.vector.reciprocal(inv8, s8)

            ident = singles.tile([128, 128], F32)
            nc.sync.dma_start(out=ident, in_=id_d[:])
            ones_sb = singles.tile([1, 128], F32)
            nc.vector.memset(ones_sb, 1.0)
            ones_f32 = singles.tile([128, R], F32)
            nc.vector.memset(ones_f32, 1.0)
            ones_tall = singles.tile([128, R], F32R)
            nc.scalar.activation(ones_tall, ones_f32, AF.Copy)
            onesq = singles.tile([R, 2, 2], F32)
            nc.vector.memset(onesq, 1.0)
            ones4 = singles.tile([128, 2, 2, 1], F32)
            nc.vector.memset(ones4, 1.0)
            ones5 = singles.tile([5, 1], F32)
            nc.vector.memset(ones5, 1.0)
            res_sb = singles.tile([1, BL], F32)

            pt = pfp.tile([1, BL], F32, tag="pff")
            nc.tensor.transpose(pt, inv8, ident[0:BL, 0:BL])
            invrow = small.tile([1, BL], F32, tag="invrow")
            nc.vector.tensor_copy(invrow, pt)
            pib = pfp.tile([128, BL], F32, tag="pff")
            nc.tensor.matmul(pib, ones_sb, invrow)
            invb4 = singles.tile([128, BL], F32)
            nc.vector.tensor_copy(invb4, pib)

            # ---- weights ----
            wqk_sb = singles.tile([128, KC, 128], F32R)
            nc.sync.dma_start(out=wqk_sb, in_=wqk_d[:].rearrange("(c p) m -> p c m", p=128))
            wv_sb = singles.tile([128, KC, R], F32R)
            nc.sync.dma_start(out=wv_sb, in_=wv_d[:].rearrange("(c p) m -> p c m", p=128))
            wqk1_sb = singles.tile([R, 128], F32R)
            nc.sync.dma_start(out=wqk1_sb, in_=wqk1_d[:])
            wv1_sb = singles.tile([R, R], F32R)
            nc.sync.dma_start(out=wv1_sb, in_=wv1_d[:])
            wqk5_sb = singles.tile([R, 128], F32R)
            nc.sync.dma_start(out=wqk5_sb, in_=wqk5_d[:])
            wv5_sb = singles.tile([R, R], F32R)
            nc.sync.dma_start(out=wv5_sb, in_=wv5_d[:])
            ao_sb = singles.tile([R, R], F32R)
            nc.sync.dma_start(out=ao_sb, in_=ao_d[:])
            ao1_sb = singles.tile([R, R], F32R)
            nc.sync.dma_start(out=ao1_sb, in_=ao1_d[:])
            ao5_sb = singles.tile([R, H], F32R)
            nc.sync.dma_start(out=ao5_sb, in_=ao5_d[:])
            g_sb = singles.tile([H, 5], F32R)
            nc.sync.dma_start(out=g_sb, in_=g_d[:])


            def attn_tail(pqk, pv, lay, ab, inv_aps):
                """pqk: [128, 2, NA] psum (pair) of raw [Aq m; Ak m];
                pv: [R, 2, NA] psum. Returns o_sb [R+1, 2, NA] f32r."""
                tq = work.tile([R, 2, NA + 2], F32R, tag="tqw")
                nc.scalar.activation(tq[:, :, 0:NA], pqk[0:R, :, :], AF.Tanh,
                                     scale=0.5)
                nc.scalar.activation(tq[:, :, NA:NA + 2], onesq, AF.Copy)
                tk = work.tile([R, 2, NA], F32R, tag="tkw")
                nc.scalar.activation(tk, pqk[R:128, :, :], AF.Tanh, scale=0.5)
                e_sb = work.tile([128, 2, 2, NA], F32R, tag="ew")
                for j in range(2):
                    for mc in range(2):
                        ps = pa.tile([128, NA + 2], F32, tag="pax")
                        nc.tensor.matmul(ps, tk[:, j, mc * 128:(mc + 1) * 128],
                                         tq[:, j, :])
                        bias_sb = small.tile([128, 1], F32, tag="bias")
                        nc.vector.tensor_scalar(bias_sb, ps[:, NA:NA + 1],
                                                inv_aps[j], None, ALU.mult)
                        nc.scalar.activation(e_sb[:, j, mc, :], ps[:, 0:NA],
                                             AF.Exp, bias=bias_sb,
                                             scale=inv_aps[j])
                tv = work.tile([R, 2, NA], F32, tag="tvw")
                nc.scalar.activation(tv, pv, AF.Tanh, scale=0.5)
                v = work.tile([128, 2, 2, R + 1], F32R, tag="vw")
                nc.scalar.activation(v[:, :, :, R:R + 1], ones4, AF.Copy)
                pvt = pb.tile([128, 2, 2, R], F32, tag="pbx")
                for j in range(2):
                    for mc in range(2):
                        nc.tensor.transpose(pvt[:, mc, j, :],
                                            tv[:, j, mc * 128:(mc + 1) * 128],
                                            ident[0:R, 0:R])
                nc.vector.tensor_scalar_add(v[:, :, :, 0:R], pvt, 1.0)
                po = pb.tile([R + 1, 2, NA], F32, tag="pbx")
                for j in range(2):
                    for mc in range(2):
                        nc.tensor.matmul(po[:, j, :], v[:, mc, j, :],
                                         e_sb[:, j, mc, :],
                                         start=(mc == 0), stop=(mc == 1))
                o_sb = work.tile([R + 1, 2, NA], F32R, tag="ow")
                nc.vector.tensor_copy(o_sb, po)
                return o_sb

            def proj_norm(o_sb, aoT_l, lay):
                """silu((Ao@o)/Z) via y'=(0.25Ao@P)*(1/Z); m = y'*(1+tanh y')"""
                if lay >= 1:
                    pm = pfp.tile([R, 2, NA], F32, tag="pff")
                else:
                    pm = pb.tile([R, 2, NA], F32, tag="pbx")
                if lay == 2:
                    pzr = pfp.tile([R, 2, NA], F32, tag="pff")
                else:
                    pzr = pa.tile([R, 2, NA], F32, tag="pax")
                nc.tensor.matmul(pm, aoT_l, o_sb[0:R, :, :])
                nc.tensor.matmul(pzr, ones_tall[R:R + 1, 0:R],
                                 o_sb[R:R + 1, :, :])
                zrec = work.tile([R, 2, NA], F32, tag="zrw")
                nc.vector.reciprocal(zrec, pzr)
                mm = work.tile([R, 2, NA], F32, tag="mmw")
                nc.vector.tensor_mul(mm, pm, zrec)
                t_sb = work.tile([R, 2, NA], F32, tag="tw")
                nc.scalar.activation(t_sb, mm, AF.Tanh)
                m_sb = work.tile([R, 2, NA], F32R, tag="mw")
                nc.vector.scalar_tensor_tensor(m_sb, t_sb, 1.0, mm,
                                               ALU.add, ALU.mult)
                return m_sb

            def stage_a(pr):
                x_sb = xpool.tile([128, KC, 2, NA], F32R, tag="x")
                nc.sync.dma_start(out=x_sb, in_=x_d[pr])
                pqk = pa.tile([128, 2, NA], F32, tag="pax")
                for k in range(KC):
                    nc.tensor.matmul(pqk, wqk_sb[:, k, :], x_sb[:, k, :, :],
                                     start=(k == 0), stop=(k == KC - 1))
                pv = pb.tile([R, 2, NA], F32, tag="pbx")
                for k in range(KC):
                    nc.tensor.matmul(pv, wv_sb[:, k, :], x_sb[:, k, :, :],
                                     start=(k == 0), stop=(k == KC - 1))
                return x_sb, pqk, pv

            def stage_b(pr, x_sb, pqk, pv):
                ab = pr % 2
                b0 = pr * 2
                inv_aps = [invb4[:, b0:b0 + 1], invb4[:, b0 + 1:b0 + 2]]
                o1 = attn_tail(pqk, pv, 0, ab, inv_aps)
                m1 = proj_norm(o1, ao_sb, 0)

                pqk2 = pa.tile([128, 2, NA], F32, tag="pax")
                nc.tensor.matmul(pqk2, wqk1_sb, m1)
                pv2 = pb.tile([R, 2, NA], F32, tag="pbx")
                nc.tensor.matmul(pv2, wv1_sb, m1)
                o2 = attn_tail(pqk2, pv2, 1, ab, inv_aps)
                m2 = proj_norm(o2, ao1_sb, 1)

                pqk3 = pa.tile([128, 2, NA], F32, tag="pax")
                nc.tensor.matmul(pqk3, wqk5_sb, m2)
                pv3 = pb.tile([R, 2, NA], F32, tag="pbx")
                nc.tensor.matmul(pv3, wv5_sb, m2)
                o3 = attn_tail(pqk3, pv3, 2, ab, inv_aps)
                m4 = proj_norm(o3, ao5_sb, 2)

                # ---- epilogue (paired where possible) ----
                m4sq = work.tile([H, 2, NA], F32R, tag="m4sq")
                nc.vector.tensor_mul(m4sq, m4, m4)
                pg = pfp.tile([5, 2, NA], F32, tag="pff")
                nc.tensor.matmul(pg, g_sb, m4sq)
                sg = small.tile([5, 2, NA], F32, tag="sg")
                nc.scalar.activation(sg, pg, AF.Copy)
                for j in range(2):
                    b = b0 + j
                    se = small.tile([4, 4 * 128], F32, tag=f"se{j}")
                    sg_pairs = sg[0:4, j, :].rearrange("p (t e) -> p e t", e=2)
                    for c in range(4):
                        off = 1 if c >= 2 else 0
                        nc.sync.dma_start(out=se[c:c + 1, :],
                                          in_=sg_pairs[:, off:off + 1, :])
                    x4a = x_sb[0:4, 0, j, 0:128]
                    x4b = x_sb[0:4, 0, j, 128:256]
                    pp = small.tile([4, 4 * 128], F32, tag=f"pp{j}")
                    nc.vector.tensor_mul(pp[:, 0:128], x4a, x4a)
                    nc.vector.tensor_mul(pp[:, 128:256], x4a, x4b)
                    nc.vector.tensor_copy(pp[:, 256:384], pp[:, 128:256])
                    nc.vector.tensor_mul(pp[:, 384:512], x4b, x4b)
                    fin = small.tile([5, 1], F32, tag=f"fin{j}")
                    nc.vector.reduce_sum(fin, sg[:, j, :],
                                         axis=mybir.AxisListType.X)
                    scr = small.tile([4, 4 * 128], F32, tag=f"scr{j}")
                    nc.vector.tensor_mul(scr, pp, se)
                    nc.vector.reduce_sum(fin[0:4, :], scr,
                                         axis=mybir.AxisListType.X)
                    pfin = pfp.tile([1, 1], F32, tag="pff")
                    nc.tensor.matmul(pfin, fin, ones5)
                    nc.vector.tensor_copy(res_sb[:, b:b + 1], pfin)

            for pr in range(NP):
                stage_b(pr, *stage_a(pr))

            nc.sync.dma_start(out=out_d[:], in_=res_sb)

    nc.finalize()
    return nc


def host_prep(inputs):
    """Split full inputs into 8 per-core input maps."""
    x = np.ascontiguousarray(inputs["x"], dtype=np.float32)
    L = np.asarray(inputs["L"], dtype=np.float32)
    w = {
        "wqk": np.ascontiguousarray(np.concatenate([inputs["Aq"].T, inputs["Ak"].T], 1), np.float32),
        "wv": np.ascontiguousarray(inputs["Av"].T, np.float32),
        "wqk1": np.ascontiguousarray(np.concatenate([inputs["Aq1"].T, inputs["Ak1"].T], 1), np.float32),
        "wv1": np.ascontiguousarray(inputs["Av1"].T, np.float32),
        "wqk5": np.ascontiguousarray(np.concatenate([inputs["Aq5"].T, inputs["Ak5"].T], 1), np.float32),
        "wv5": np.ascontiguousarray(inputs["Av5"].T, np.float32),
        "aoT": np.ascontiguousarray(0.25 * inputs["Ao"].T, np.float32),
        "ao1T": np.ascontiguousarray(0.25 * inputs["Ao1"].T, np.float32),
        "ao5T": np.ascontiguousarray(0.25 * inputs["Ao5"].T, np.float32),
    }
    G = np.zeros((H, 5), np.float32)
    for row in range(H):
        g = row // 8
        G[row, g if g < 4 else 4] = 1.0
    w["gmat"] = G
    w["ident"] = np.eye(128, dtype=np.float32)
    in_maps = []
    for core in range(NCORES):
        sl = slice(core * BL, (core + 1) * BL)
        m = dict(w)
        xc = x[sl].reshape(NP, 2, KC, 128, NA).transpose(0, 3, 2, 1, 4)
        m["x"] = np.ascontiguousarray(xc)
        m["l0"] = np.ascontiguousarray(L[sl, 0, :])
        in_maps.append(m)
    return in_maps


def kernel_run(inputs, trace=False):
    if "nc" not in _cache:
        _cache["nc"] = build_bass()
    nc = _cache["nc"]
    in_maps = host_prep(inputs)
    res = run_bass_kernel_spmd(nc, in_maps, core_ids=list(range(NCORES)),
                               trace=trace)
    outs = [res.results[c]["out"].reshape(BL, 1) for c in range(NCORES)]
    full = np.concatenate(outs, 0).astype(np.float32)
    return full, res.exec_time_ns


def kernel(**inputs):
    out, _ = kernel_run(inputs, trace=False)
    return out


# revision 59
# speedup vs baseline: 1.4233x; 1.1601x over previous
"""Trainium2 Bass kernel for nn_Att_H_47571057771075.

Data-parallel over batch: 64 batches -> 8 cores x 8 batches; on each core the
8 batches are processed as 4 pairs so every elementwise/matmul instruction
covers two batches (amortizes per-instruction fixed costs).

Performance design:
- float32r end-to-end for every matmul operand (1 cycle/row on the PE for
  moving dim >= 256, vs 4 for plain fp32).
- One ACT table set for the whole kernel (exp_and_others): sigmoid/silu come
  from tanh, softmax from exp; only the one-time numNeighbors sqrt uses
  another set.
- Q and K share one PE pass (stationary [Wq^T|Wk^T]); scores are computed
  transposed so exp runs straight off PSUM and A@V needs no transpose of A.
- sigmoid(x) = 0.5(1+tanh(x/2)): the +1 rank-1 terms of Q,K fold into an
  extra all-ones column of the scores matmul (per-key bias into exp) and are
  dropped for the per-query terms (softmax-invariant). Constant 0.5/0.25
  factors fold into host-side weights / the exp scale.
- Softmax denominator Z comes free from an all-ones stationary column in the
  A@V matmul; normalization is deferred through the linear output projection.
- Ones-columns live in persistent double-buffered tiles, written once.
"""
import os
import sys

for p in ("/opt/trn_rl_repo",):
    if p not in sys.path:
        sys.path.insert(0, p)

import numpy as np

import concourse.bacc as bacc
import concourse.tile as tile
from concourse import mybir
from concourse.masks import make_identity
from concourse.bass_utils import run_bass_kernel_spmd

F32 = mybir.dt.float32
F32R = mybir.dt.float32r
AF = mybir.ActivationFunctionType
ALU = mybir.AluOpType

B, D, NA, R, H = 64, 2048, 256, 64, 64
NCORES = 8
BL = B // NCORES          # batches per core
NP = BL // 2              # batch pairs per core
KC = D // 128             # k-chunks for layer 1 (16)

_cache = {}


def build_bass():
    nc = bacc.Bacc("TRN2", target_bir_lowering=False, debug=False)

    x_d = nc.dram_tensor("x", [NP, 128, KC, 2, NA], F32R, kind="ExternalInput")
    l_d = nc.dram_tensor("l0", [BL, NA], F32, kind="ExternalInput")
    wqk_d = nc.dram_tensor("wqk", [D, 128], F32R, kind="ExternalInput")
    wv_d = nc.dram_tensor("wv", [D, R], F32R, kind="ExternalInput")
    wqk1_d = nc.dram_tensor("wqk1", [R, 128], F32R, kind="ExternalInput")
    wv1_d = nc.dram_tensor("wv1", [R, R], F32R, kind="ExternalInput")
    wqk5_d = nc.dram_tensor("wqk5", [R, 128], F32R, kind="ExternalInput")
    wv5_d = nc.dram_tensor("wv5", [R, R], F32R, kind="ExternalInput")
    ao_d = nc.dram_tensor("aoT", [R, R], F32R, kind="ExternalInput")
    ao1_d = nc.dram_tensor("ao1T", [R, R], F32R, kind="ExternalInput")
    ao5_d = nc.dram_tensor("ao5T", [R, H], F32R, kind="ExternalInput")
    g_d = nc.dram_tensor("gmat", [H, 5], F32R, kind="ExternalInput")
    id_d = nc.dram_tensor("ident", [128, 128], F32, kind="ExternalInput")
    out_d = nc.dram_tensor("out", [1, BL], F32, kind="ExternalOutput")

    with tile.TileContext(nc) as tc:
        with (
            tc.tile_pool(name="singles", bufs=1) as singles,
            tc.tile_pool(name="xpool", bufs=2) as xpool,
            tc.tile_pool(name="work", bufs=3) as work,
            tc.tile_pool(name="small", bufs=3) as small,
            tc.tile_pool(name="pa", bufs=3, space="PSUM") as pa,
            tc.tile_pool(name="pb", bufs=3, space="PSUM") as pb,
            tc.tile_pool(name="pf", bufs=2, space="PSUM") as pfp,
        ):
            wqk_sb = singles.tile([128, KC, 128], F32R)
            nc.sync.dma_start(out=wqk_sb, in_=wqk_d[:].rearrange("(c p) m -> p c m", p=128))
            wv_sb = singles.tile([128, KC, R], F32R)
            nc.sync.dma_start(out=wv_sb, in_=wv_d[:].rearrange("(c p) m -> p c m", p=128))

            # ---- per-batch 0.25/sqrt(numN) first (one sqrt-table load) ----
            l_sb = small.tile([BL, NA], F32, tag="lsb")
            nc.sync.dma_start(out=l_sb, in_=l_d[:])
            ind = small.tile([BL, NA], F32, tag="ind")
            nc.vector.tensor_single_scalar(ind, l_sb, 1.0, ALU.is_ge)
            s8 = small.tile([BL, 1], F32, tag="s8")
            nc.vector.reduce_sum(s8, ind, axis=mybir.AxisListType.X)
            c16 = small.tile([BL, 1], F32, tag="c16")
            nc.vector.memset(c16, 16.0)
            nc.scalar.activation(s8, s8, AF.Sqrt, bias=c16, scale=16.0)
            inv8 = small.tile([BL, 1], F32, tag="inv8")
            nc.vector.reciprocal(inv8, s8)

            ident = singles.tile([128, 128], F32)
            nc.sync.dma_start(out=ident, in_=id_d[:])
            # PE warm-up during the first x DMA: ramps the PE out of its cold
            # p-state before the first real QK block arrives.
            junk = pb.tile([128, 128], F32, tag="pbx")
            for _ in range(12):
                nc.tensor.matmul(junk, ident, ident)
            ones_sb = singles.tile([1, 128], F32)
            nc.vector.memset(ones_sb, 1.0)
            ones_f32 = singles.tile([128, R], F32)
            nc.vector.memset(ones_f32, 1.0)
            ones_tall = singles.tile([128, R], F32R)
            nc.scalar.activation(ones_tall, ones_f32, AF.Copy)
            onesq = singles.tile([R, 2, 2], F32)
            nc.vector.memset(onesq, 1.0)
            ones4 = singles.tile([128, 2, 2, 1], F32)
            nc.vector.memset(ones4, 1.0)
            ones5 = singles.tile([5, 1], F32)
            nc.vector.memset(ones5, 1.0)
            res_sb = singles.tile([1, BL], F32)

            pt = pfp.tile([1, BL], F32, tag="pff")
            nc.tensor.transpose(pt, inv8, ident[0:BL, 0:BL])
            invrow = small.tile([1, BL], F32, tag="invrow")
            nc.vector.tensor_copy(invrow, pt)
            pib = pfp.tile([128, BL], F32, tag="pff")
            nc.tensor.matmul(pib, ones_sb, invrow)
            invb4 = singles.tile([128, BL], F32)
            nc.vector.tensor_copy(invb4, pib)

            # ---- weights ----
            wqk1_sb = singles.tile([R, 128], F32R)
            nc.sync.dma_start(out=wqk1_sb, in_=wqk1_d[:])
            wv1_sb = singles.tile([R, R], F32R)
            nc.sync.dma_start(out=wv1_sb, in_=wv1_d[:])
            wqk5_sb = singles.tile([R, 128], F32R)
            nc.sync.dma_start(out=wqk5_sb, in_=wqk5_d[:])
            wv5_sb = singles.tile([R, R], F32R)
            nc.sync.dma_start(out=wv5_sb, in_=wv5_d[:])
            ao_sb = singles.tile([R, R], F32R)
            nc.sync.dma_start(out=ao_sb, in_=ao_d[:])
            ao1_sb = singles.tile([R, R], F32R)
            nc.sync.dma_start(out=ao1_sb, in_=ao1_d[:])
            ao5_sb = singles.tile([R, H], F32R)
            nc.sync.dma_start(out=ao5_sb, in_=ao5_d[:])
            g_sb = singles.tile([H, 5], F32R)
            nc.sync.dma_start(out=g_sb, in_=g_d[:])


            def attn_tail(pqk, pv, lay, ab, inv_aps):
                """pqk: [128, 2, NA] psum (pair) of raw [Aq m; Ak m];
                pv: [R, 2, NA] psum. Returns o_sb [R+1, 2, NA] f32r."""
                tq = work.tile([R, 2, NA + 2], F32R, tag="tqw")
                nc.scalar.activation(tq[:, :, 0:NA], pqk[0:R, :, :], AF.Tanh,
                                     scale=0.5)
                nc.vector.tensor_copy(tq[:, :, NA:NA + 2], onesq)
                tk = work.tile([R, 2, NA], F32R, tag="tkw")
                nc.scalar.activation(tk, pqk[R:128, :, :], AF.Tanh, scale=0.5)
                e_sb = work.tile([128, 2, 2, NA], F32R, tag="ew")
                for j in range(2):
                    for mc in range(2):
                        ps = pa.tile([128, NA + 2], F32, tag="pax")
                        nc.tensor.matmul(ps, tk[:, j, mc * 128:(mc + 1) * 128],
                                         tq[:, j, :])
                        bias_sb = small.tile([128, 1], F32, tag="bias")
                        nc.vector.tensor_scalar(bias_sb, ps[:, NA:NA + 1],
                                                inv_aps[j], None, ALU.mult)
                        nc.scalar.activation(e_sb[:, j, mc, :], ps[:, 0:NA],
                                             AF.Exp, bias=bias_sb,
                                             scale=inv_aps[j])
                tv = work.tile([R, 2, NA], F32, tag="tvw")
                nc.scalar.activation(tv, pv, AF.Tanh, scale=0.5)
                v = work.tile([128, 2, 2, R + 1], F32R, tag="vw")
                nc.vector.tensor_copy(v[:, :, :, R:R + 1], ones4)
                pvt = pb.tile([128, 2, 2, R], F32, tag="pbx")
                for j in range(2):
                    for mc in range(2):
                        nc.tensor.transpose(pvt[:, mc, j, :],
                                            tv[:, j, mc * 128:(mc + 1) * 128],
                                            ident[0:R, 0:R])
                nc.vector.tensor_scalar_add(v[:, :, :, 0:R], pvt, 1.0)
                po = pb.tile([R + 1, 2, NA], F32, tag="pbx")
                for j in range(2):
                    for mc in range(2):
                        nc.tensor.matmul(po[:, j, :], v[:, mc, j, :],
                                         e_sb[:, j, mc, :],
                                         start=(mc == 0), stop=(mc == 1))
                o_sb = work.tile([R + 1, 2, NA], F32R, tag="ow")
                nc.vector.tensor_copy(o_sb, po)
                return o_sb

            def proj_norm(o_sb, aoT_l, lay):
                """silu((Ao@o)/Z) via y'=(0.25Ao@P)*(1/Z); m = y'*(1+tanh y')"""
                if lay >= 1:
                    pm = pfp.tile([R, 2, NA], F32, tag="pff")
                else:
                    pm = pb.tile([R, 2, NA], F32, tag="pbx")
                if lay == 2:
                    pzr = pfp.tile([R, 2, NA], F32, tag="pff")
                else:
                    pzr = pa.tile([R, 2, NA], F32, tag="pax")
                nc.tensor.matmul(pm, aoT_l, o_sb[0:R, :, :])
                nc.tensor.matmul(pzr, ones_tall[R:R + 1, 0:R],
                                 o_sb[R:R + 1, :, :])
                zrec = work.tile([R, 2, NA], F32, tag="zrw")
                nc.vector.reciprocal(zrec, pzr)
                mm = work.tile([R, 2, NA], F32, tag="mmw")
                nc.vector.tensor_mul(mm, pm, zrec)
                t_sb = work.tile([R, 2, NA], F32, tag="tw")
                nc.scalar.activation(t_sb, mm, AF.Tanh)
                m_sb = work.tile([R, 2, NA], F32R, tag="mw")
                nc.vector.scalar_tensor_tensor(m_sb, t_sb, 1.0, mm,
                                               ALU.add, ALU.mult)
                return m_sb

            def stage_a(pr):
                x_sb = xpool.tile([128, KC, 2, NA], F32R, tag="x")
                for q in range(8):
                    nc.sync.dma_start(out=x_sb[:, 2 * q:2 * q + 2, :, :],
                                      in_=x_d[pr, :, 2 * q:2 * q + 2, :, :])
                pqk = pa.tile([128, 2, NA], F32, tag="pax")
                for k in range(KC):
                    nc.tensor.matmul(pqk, wqk_sb[:, k, :], x_sb[:, k, :, :],
                                     start=(k == 0), stop=(k == KC - 1))
                pv = pb.tile([R, 2, NA], F32, tag="pbx")
                for k in range(KC):
                    nc.tensor.matmul(pv, wv_sb[:, k, :], x_sb[:, k, :, :],
                                     start=(k == 0), stop=(k == KC - 1))
                return x_sb, pqk, pv

            def stage_b(pr, x_sb, pqk, pv):
                ab = pr % 2
                b0 = pr * 2
                inv_aps = [invb4[:, b0:b0 + 1], invb4[:, b0 + 1:b0 + 2]]
                o1 = attn_tail(pqk, pv, 0, ab, inv_aps)
                m1 = proj_norm(o1, ao_sb, 0)

                pqk2 = pa.tile([128, 2, NA], F32, tag="pax")
                nc.tensor.matmul(pqk2, wqk1_sb, m1)
                pv2 = pb.tile([R, 2, NA], F32, tag="pbx")
                nc.tensor.matmul(pv2, wv1_sb, m1)
                o2 = attn_tail(pqk2, pv2, 1, ab, inv_aps)
                m2 = proj_norm(o2, ao1_sb, 1)

                pqk3 = pa.tile([128, 2, NA], F32, tag="pax")
                nc.tensor.matmul(pqk3, wqk5_sb, m2)
                pv3 = pb.tile([R, 2, NA], F32, tag="pbx")
                nc.tensor.matmul(pv3, wv5_sb, m2)
                o3 = attn_tail(pqk3, pv3, 2, ab, inv_aps)
                m4 = proj_norm(o3, ao5_sb, 2)

                # ---- epilogue (paired where possible) ----
                m4sq = work.tile([H, 2, NA], F32R, tag="m4sq")
                nc.vector.tensor_mul(m4sq, m4, m4)
                pg = pfp.tile([5, 2, NA], F32, tag="pff")
                nc.tensor.matmul(pg, g_sb, m4sq)
                sg = small.tile([5, 2, NA], F32, tag="sg")
                nc.vector.tensor_copy(sg, pg)
                for j in range(2):
                    b = b0 + j
                    se = small.tile([4, 4 * 128], F32, tag=f"se{j}")
                    sg_pairs = sg[0:4, j, :].rearrange("p (t e) -> p e t", e=2)
                    for c in range(4):
                        off = 1 if c >= 2 else 0
                        nc.sync.dma_start(out=se[c:c + 1, :],
                                          in_=sg_pairs[:, off:off + 1, :])
                    x4a = x_sb[0:4, 0, j, 0:128]
                    x4b = x_sb[0:4, 0, j, 128:256]
                    pp = small.tile([4, 4 * 128], F32, tag=f"pp{j}")
                    nc.vector.tensor_mul(pp[:, 0:128], x4a, x4a)
                    nc.vector.tensor_mul(pp[:, 128:256], x4a, x4b)
                    nc.vector.tensor_copy(pp[:, 256:384], pp[:, 128:256])
                    nc.vector.tensor_mul(pp[:, 384:512], x4b, x4b)
                    fin = small.tile([5, 1], F32, tag=f"fin{j}")
                    nc.vector.reduce_sum(fin, sg[:, j, :],
                                         axis=mybir.AxisListType.X)
                    scr = small.tile([4, 4 * 128], F32, tag=f"scr{j}")
                    nc.vector.tensor_mul(scr, pp, se)
                    nc.vector.reduce_sum(fin[0:4, :], scr,
                                         axis=mybir.AxisListType.X)
                    pfin = pfp.tile([1, 1], F32, tag="pff")
                    nc.tensor.matmul(pfin, fin, ones5)
                    nc.vector.tensor_copy(res_sb[:, b:b + 1], pfin)

            for pr in range(NP):
                stage_b(pr, *stage_a(pr))

            nc.sync.dma_start(out=out_d[:], in_=res_sb)

    nc.finalize()
    return nc


def host_prep(inputs):
    """Split full inputs into 8 per-core input maps."""
    x = np.ascontiguousarray(inputs["x"], dtype=np.float32)
    L = np.asarray(inputs["L"], dtype=np.float32)
    w = {
        "wqk": np.ascontiguousarray(np.concatenate([inputs["Aq"].T, inputs["Ak"].T], 1), np.float32),
        "wv": np.ascontiguousarray(inputs["Av"].T, np.float32),
        "wqk1": np.ascontiguousarray(np.concatenate([inputs["Aq1"].T, inputs["Ak1"].T], 1), np.float32),
        "wv1": np.ascontiguousarray(inputs["Av1"].T, np.float32),
        "wqk5": np.ascontiguousarray(np.concatenate([inputs["Aq5"].T, inputs["Ak5"].T], 1), np.float32),
        "wv5": np.ascontiguousarray(inputs["Av5"].T, np.float32),
        "aoT": np.ascontiguousarray(0.25 * inputs["Ao"].T, np.float32),
        "ao1T": np.ascontiguousarray(0.25 * inputs["Ao1"].T, np.float32),
        "ao5T": np.ascontiguousarray(0.25 * inputs["Ao5"].T, np.float32),
    }
    G = np.zeros((H, 5), np.float32)
    for row in range(H):
        g = row // 8
        G[row, g if g < 4 else 4] = 1.0
    w["gmat"] = G
    w["ident"] = np.eye(128, dtype=np.float32)
    in_maps = []
    for core in range(NCORES):
        sl = slice(core * BL, (core + 1) * BL)
        m = dict(w)
        xc = x[sl].reshape(NP, 2, KC, 128, NA).transpose(0, 3, 2, 1, 4)
        m["x"] = np.ascontiguousarray(xc)
        m["l0"] = np.ascontiguousarray(L[sl, 0, :])
        in_maps.append(m)
    return in_maps


def kernel_run(inputs, trace=False):
    if "nc" not in _cache:
        _cache["nc"] = build_bass()
    nc = _cache["nc"]
    in_maps = host_prep(inputs)
    res = run_bass_kernel_spmd(nc, in_maps, core_ids=list(range(NCORES)),
                               trace=trace)
    outs = [res.results[c]["out"].reshape(BL, 1) for c in range(NCORES)]
    full = np.concatenate(outs, 0).astype(np.float32)
    return full, res.exec_time_ns


def kernel(**inputs):
    out, _ = kernel_run(inputs, trace=False)
    return out
